# revision 35
# baseline (speedup 1.0000x reference)
"""Trainium2 Bass kernel for nn_InputRotationWrapper: y = WHT(x) @ W^T + b.

Algebraic fold: WHT (normalized Walsh-Hadamard along feature dim, H symmetric)
commutes into the weight: y = (x H) W^T = x (W H)^T.  The device therefore runs
a pure GEMM  y = x @ Wr^T + b  with Wr = WHT(W) computed once on the host.

Distribution: data-parallel over the 8192 tokens across 8 NeuronCores (1024
tokens each); Wr is replicated.  Each core computes its output slice
transposed (yT[o, t], o on partitions) so every DMA is fully contiguous:
  - x^T shard  [128 p, 32 c, 1024 t]  resident in SBUF as fp16 (8.4 MB)
  - Wr packed  [32 ob, 128 d_in, 32 d_chunk, 128 o]  fp16, streamed per o-block
  - out yT     [4096 o, 1024 t]  fp16, written per o-block (host re-widens)

Matmul dtype is float16: full PE rate (1 row/cycle), and the per-matmul
LDWEIGHTS (128x128 stationary tile) takes ~100ns at 2B/row vs fp32r's 224ns,
so it hides completely in the PE shadow weight buffer behind the previous
512-row matmul (213ns).  fp32r's 224+44ns load path gated the original
kernel at a 272ns cadence; fp16 runs at the ~216ns roofline cadence.

Startup is HBM-arrival-bound (the DMA subsystem ramps from ~85 GB/s to
~400 GB/s over the first few us): x streams on the Scalar HWDGE queue in
size-ramped pieces while the warmup W quarters stream on Sync, both in
arrival-need order.  The warmup group runs c-outer over 4 o-blocks (all 8
PSUM banks) so each arriving x chunk immediately unlocks 8 matmuls, and
finishes block-major so its evictions stagger.  Bias is fused into the
PSUM->SBUF eviction via ScalarE activation.  The last o-block runs
token-half 0 fully before half 1 (separate PSUM tiles — a shared tile
would serialize on a whole-tile WAR edge) so only one half-eviction tails.
"""
import sys

for _p in ("/opt/trn_rl_repo", "/root/.axon_site/_ro/trn_rl_repo"):
    if _p not in sys.path:
        sys.path.insert(0, _p)

import numpy as np

D = 4096          # feature dim (= rotation size)
TOKENS = 8192     # 4 * 2048
N_CORES = 8
T_CORE = TOKENS // N_CORES   # 1024 tokens per core
P = 128           # partitions
DC = D // P       # 32 contraction chunks
OB = D // P       # 32 output blocks
T_HALF = 512      # moving free-dim per matmul (hw max)

_compiled = None


def _matmul_hadU_np(x: np.ndarray) -> np.ndarray:
    """Normalized WHT along the last axis — exact port of the reference
    recursive-butterfly (K == 1 branch), in float64."""
    n = x.shape[-1]
    shape = x.shape
    v = x.reshape(-1, n, 1)
    while v.shape[1] > 1:
        b_, m, c = v.shape
        v = v.reshape(b_, m // 2, 2, c)
        a, b = v[:, :, 0, :], v[:, :, 1, :]
        v = np.concatenate([a + b, a - b], axis=-1)
    return v.reshape(shape) / np.sqrt(n)


def _build_nc():
    import concourse.tile as tile
    from concourse import bacc, mybir

    dt = mybir.dt
    nc = bacc.Bacc(None, target_bir_lowering=False)

    xt_d = nc.dram_tensor("xt", [P, DC, T_CORE], dt.float16, kind="ExternalInput")
    w_d = nc.dram_tensor("w", [OB, P, DC, P], dt.float16, kind="ExternalInput")
    b_d = nc.dram_tensor("bias", [P, OB], dt.float32, kind="ExternalInput")
    y_d = nc.dram_tensor("yt", [D, T_CORE], dt.float16, kind="ExternalOutput")

    G0 = 4   # o-blocks processed c-outer in the startup group: 8 matmuls
             # become ready per arriving x chunk, saturating the PE while the
             # 8.4 MB x shard streams in.  Uses all 8 PSUM banks.
    QC = 8   # startup W granularity: quarter-tiles of 8 contraction chunks
    HC = 16  # steady-state W granularity: half-tiles (fewer DMA triggers)
    NQ = DC // QC
    PRE = 3  # steady blocks whose W is prefetched on Sync during startup

    with tile.TileContext(nc) as tc:
        with (
            tc.tile_pool(name="xp", bufs=1) as xp,
            tc.tile_pool(name="wqp", bufs=G0 * NQ, space="SBUF") as wqp,
            tc.tile_pool(name="whp", bufs=2 * (PRE + 1), space="SBUF") as whp,
            tc.tile_pool(name="bp", bufs=1) as bp,
            tc.tile_pool(name="op", bufs=4) as op,
            tc.tile_pool(name="pp", bufs=G0, space="PSUM") as pp,
        ):
            b_sb = bp.tile([P, OB], dt.float32)

            ps0 = [
                pp.tile([P, T_CORE], dt.float32, tag="ps", name=f"ps0_{i}")
                for i in range(G0)
            ]

            # PE warm-up: the PE clock p-state drops after ~1-2us idle and
            # takes ~3us of activity to ramp back, and the engines sit idle
            # from the end of the preamble (~7.3us) until the first x/W
            # bytes land (~12.2us).  Fill that whole dead window with dummy
            # matmuls (~213ns each) into a PSUM region the real c=0
            # accumulation resets (start=True), so the first real matmuls
            # run at full clock instead of ramping on the critical path.
            dum = bp.tile([P, 256], dt.float16, tag="dum", name="dum")
            nc.vector.memset(dum[:], 0.0)
            for _ in range(23):
                nc.tensor.matmul(
                    ps0[0][:, 0:256], dum[:, 0:128], dum[:, 0:256],
                    start=True, stop=True,
                )

            def load_wq(ob, q):
                wq = wqp.tile([P, QC, P], dt.float16, tag="w",
                              name=f"w_{ob}_{q}")
                nc.sync.dma_start(wq[:], w_d[ob, :, q * QC:(q + 1) * QC, :])
                return wq

            def load_wh(ob, h):
                wh = whp.tile([P, HC, P], dt.float16, tag="w2", name=f"wh_{ob}_{h}")
                nc.sync.dma_start(wh[:], w_d[ob, :, h * HC:(h + 1) * HC, :])
                return wh

            # per-chunk (half0_ap, half1_ap) matmul operand pairs
            x_half = [None] * DC

            def load_x_group(c0, n):
                t = xp.tile([P, n, T_CORE], dt.float16, tag=f"x{c0}",
                            name=f"x_{c0}")
                nc.scalar.dma_start(t[:], xt_d[:, c0:c0 + n, :])
                for i in range(n):
                    x_half[c0 + i] = (t[:, i, 0:T_HALF],
                                      t[:, i, T_HALF:T_CORE])

            def mms(ps, lhsT, c):
                xh = x_half[c]
                nc.tensor.matmul(
                    ps[:, 0:T_HALF], lhsT, xh[0],
                    start=(c == 0), stop=(c == DC - 1),
                )
                nc.tensor.matmul(
                    ps[:, T_HALF:T_CORE], lhsT, xh[1],
                    start=(c == 0), stop=(c == DC - 1),
                )


            def evict_half(ob, ps, h):
                sl = slice(h * T_HALF, (h + 1) * T_HALF)
                o_sb = op.tile([P, T_HALF], dt.float16, tag="o",
                               name=f"o_{ob}_{h}")
                nc.scalar.activation(
                    o_sb[:], ps[:, sl],
                    mybir.ActivationFunctionType.Identity,
                    bias=b_sb[:, ob:ob + 1],
                )
                nc.sync.dma_start(y_d[ob * P:(ob + 1) * P, sl], o_sb[:])

            # Scalar HWDGE queue carries x in need order with sizes ramping
            # up; Sync carries the W quarters.  Trigger order approximates
            # arrival-need order across both queues.
            w0 = [[None] * NQ for _ in range(G0)]
            load_x_group(0, 1)
            for ob in range(G0):
                w0[ob][0] = load_wq(ob, 0)
            load_x_group(1, 1)
            load_x_group(2, 1)
            load_x_group(3, 1)
            for ob in range(G0):
                w0[ob][1] = load_wq(ob, 1)
            load_x_group(4, 2)
            load_x_group(6, 2)
            load_x_group(8, 4)
            for ob in range(G0):
                w0[ob][2] = load_wq(ob, 2)
            load_x_group(12, 4)
            load_x_group(16, 4)
            for ob in range(G0):
                w0[ob][3] = load_wq(ob, 3)
            load_x_group(20, 4)
            load_x_group(24, 4)
            load_x_group(28, 4)

            # Sync queue: bias, then W for the first steady blocks so block
            # G0 starts without waiting on the eviction-gated trigger chain.
            nc.sync.dma_start(b_sb[:], b_d[:])
            whs = {}
            for ob in range(G0, G0 + PRE):
                whs[ob] = [load_wh(ob, h) for h in range(2)]

            # c-outer while x streams in; the last 4 c-steps go block-major
            # so startup blocks finish staggered and their evictions (which
            # free the PSUM ring for the first steady blocks) pipeline with
            # the remaining matmuls instead of bunching at the end.
            C_SPLIT = DC - 4
            for c in range(C_SPLIT):
                for ob in range(G0):
                    mms(ps0[ob], w0[ob][c // QC][:, c % QC, :], c)
            for ob in range(G0):
                for c in range(C_SPLIT, DC):
                    mms(ps0[ob], w0[ob][c // QC][:, c % QC, :], c)
                evict_half(ob, ps0[ob], 0)
                evict_half(ob, ps0[ob], 1)

            # steady state: one o-block at a time, W halves prefetched
            for ob in range(G0, OB):
                if ob not in whs:
                    whs[ob] = [load_wh(ob, h) for h in range(2)]
                nxt = ob + PRE
                if G0 + PRE <= nxt < OB:
                    whs[nxt] = [load_wh(nxt, h) for h in range(2)]
                if ob < OB - 1:
                    ps = pp.tile([P, T_CORE], dt.float32, tag="ps",
                                 name=f"ps_{ob}")
                    for c in range(DC):
                        mms(ps, whs[ob][c // HC][:, c % HC, :], c)
                    evict_half(ob, ps, 0)
                    evict_half(ob, ps, 1)
                else:
                    # last block: finish token-half 0 first so its eviction
                    # overlaps half 1's matmuls; only half 1's eviction
                    # tails.  Separate PSUM tiles per half — with a shared
                    # tile the h1 matmuls pick up a whole-tile WAR edge on
                    # the h0 eviction read and stall ~1.2us.
                    for h in range(2):
                        psh = pp.tile([P, T_CORE], dt.float32, tag="ps",
                                      name=f"ps_{ob}_{h}")
                        sl = slice(h * T_HALF, (h + 1) * T_HALF)
                        for c in range(DC):
                            nc.tensor.matmul(
                                psh[:, sl], whs[ob][c // HC][:, c % HC, :],
                                x_half[c][h],
                                start=(c == 0), stop=(c == DC - 1),
                            )
                        evict_half(ob, psh, h)

    nc.compile()
    return nc


def _get_nc():
    global _compiled
    if _compiled is None:
        _compiled = _build_nc()
    return _compiled


def _prep_inputs(x, W, b):
    x = np.asarray(x, dtype=np.float32)
    W = np.asarray(W, dtype=np.float32)
    b = np.asarray(b, dtype=np.float32)

    Wr = _matmul_hadU_np(W.astype(np.float64))  # [o, d] float64
    # W_pack[ob, p, c, j] = Wr[ob*128 + j, c*128 + p]
    w_pack = np.ascontiguousarray(
        Wr.reshape(OB, P, DC, P).transpose(0, 3, 2, 1).astype(np.float16)
    )
    b_pack = np.ascontiguousarray(b.reshape(OB, P).T)  # [128, 32]

    # xt[core, p, c, t] = x_core^T[c*128 + p, t]: partition-major so each
    # multi-chunk DMA reads one contiguous span per partition.
    xt = np.ascontiguousarray(
        x.reshape(N_CORES, T_CORE, D).transpose(0, 2, 1).astype(np.float16)
        .reshape(N_CORES, DC, P, T_CORE).transpose(0, 2, 1, 3)
    )

    in_maps = [
        {"xt": xt[c], "w": w_pack, "bias": b_pack} for c in range(N_CORES)
    ]
    return in_maps


def _assemble(results):
    # yt per core: [4096 o, 1024 t] fp16 -> y[t, o] fp32
    parts = [r["yt"].T.astype(np.float32) for r in results]
    y = np.concatenate(parts, axis=0)  # [8192, 4096]
    return y.reshape(4, 2048, D)


def _run(x, W, b, **spmd_kwargs):
    from concourse.bass_utils import run_bass_kernel_spmd

    nc = _get_nc()
    in_maps = _prep_inputs(x, W, b)
    res = run_bass_kernel_spmd(nc, in_maps, list(range(N_CORES)), **spmd_kwargs)
    return _assemble(res.results), res


def kernel(x, W, b):
    out, _ = _run(x, W, b)
    return out

